# revision 14
# baseline (speedup 1.0000x reference)
"""Trainium2 Bass kernel for nn_Encoder_base (5x ChebConv GNN + pool + MLP).

Restructured for speed:
  - Only pooled rows of each ChebConv are needed downstream, so each level's
    Chebyshev taps are computed directly at the pooled nodes via
    host-precomputed operators: tap1 = S[pool,:] z, tap2 = (S^2)[pool,:] z.
  - Level 0 applies them as per-edge selection matmuls whose source rows are
    HOST-gathered from x (x is a kernel input), streamed with the weighted
    selection matrices as one bf16 stream -> zero device gathers, zero DVE.
  - Levels 1-2 apply the pooled operators as dense bf16 matmuls; level 3
    (no pooling) uses dense S3 / S3^2.
  - Everything on the matmul path is bf16 (fp32 matmul is 4 cycles/row on
    TRN2, bf16 is 1 and gets fast weight load).
  - Distribution: level 0 edge-sharded by pooled dest (one small AllGather
    of the taps); levels 1-3 batch-sharded (4 batches/core, replicated
    small dense ops); MLP output-feature sharded with per-layer AllGathers.
"""
import numpy as np
import ml_dtypes
import concourse.bass as bass
import concourse.bacc as bacc
import concourse.tile as tile
from concourse import mybir, bass_utils

F32 = mybir.dt.float32
BF16 = mybir.dt.bfloat16
I16 = mybir.dt.int16
NPBF = ml_dtypes.bfloat16
AF = mybir.ActivationFunctionType
ALU = mybir.AluOpType
AX = mybir.AxisListType
RG = [list(range(8))]
NCORES = 8
N0, N1, N2, N3 = 16384, 4096, 1024, 128
EPS = 1e-5

_CACHE = {}


# ---------------------------------------------------------------- host prep
def _edge_we(e, n):
    row = np.asarray(e[0], np.int64)
    col = np.asarray(e[1], np.int64)
    deg = np.bincount(row, minlength=n).astype(np.float32)
    dis = np.where(deg > 0, 1.0 / np.sqrt(np.maximum(deg, 1.0)), 0.0).astype(np.float32)
    return row, col, -(dis[row] * dis[col]).astype(np.float32)


def _sort_by_row(row, col, we):
    order = np.argsort(row, kind="stable")
    return row[order], col[order], we[order]


def _sub_edges(row, col, we, pool_idx):
    row, col, we = _sort_by_row(row, col, we)
    starts = np.searchsorted(row, pool_idx, side="left")
    ends = np.searchsorted(row, pool_idx, side="right")
    cnt = ends - starts
    tot = int(cnt.sum())
    pos = np.repeat(starts, cnt) + (np.arange(tot) - np.repeat(np.cumsum(cnt) - cnt, cnt))
    nr = np.repeat(np.arange(len(pool_idx), dtype=np.int64), cnt)
    return nr, col[pos], we[pos]


def _two_hop(rd, cd, wd, row, col, we, n):
    row, col, we = _sort_by_row(row, col, we)
    indptr = np.searchsorted(row, np.arange(n + 1))
    s, e = indptr[cd], indptr[cd + 1]
    cnt = e - s
    tot = int(cnt.sum())
    pos = np.repeat(s, cnt) + (np.arange(tot) - np.repeat(np.cumsum(cnt) - cnt, cnt))
    return np.repeat(rd, cnt), col[pos], np.repeat(wd, cnt) * we[pos]


def _dense_op(rd, cd, wd, n_rows, n_cols):
    m = np.zeros((n_rows, n_cols), np.float32)
    np.add.at(m, (rd, cd), wd)
    return m


def _prep_l0_streams(ops, x_rows, n_dest, window=128):
    """Per-edge (xg | sel) bf16 streams, dest-sharded over cores."""
    per = n_dest // NCORES
    nwin = per // window
    feat = x_rows.shape[1]
    ncw_per_op, lists = [], {}
    for oi, (rd, cd, wd) in enumerate(ops):
        rd, cd, wd = _sort_by_row(rd, cd, wd)
        counts = np.zeros((NCORES, nwin), np.int64)
        for k in range(NCORES):
            lo = k * per
            for wi in range(nwin):
                a = np.searchsorted(rd, lo + wi * window, side="left")
                b = np.searchsorted(rd, lo + (wi + 1) * window, side="left")
                lists[(oi, k, wi)] = (rd[a:b] - (lo + wi * window), cd[a:b], wd[a:b])
                counts[k, wi] = (b - a + 127) // 128
        ncw_per_op.append(np.maximum(counts.max(axis=0), 1).astype(np.int64))
    ctot = int(sum(n.sum() for n in ncw_per_op))
    win_of_chunk = []
    for oi in range(len(ops)):
        for wi in range(nwin):
            win_of_chunk += [wi] * int(ncw_per_op[oi][wi])
    xgs, dwes = [], []
    for k in range(NCORES):
        xg = np.zeros((ctot * 128, 128), NPBF)
        dst = np.zeros((ctot * 128,), NPBF)
        we = np.zeros((ctot * 128,), NPBF)
        cbase = 0
        for oi in range(len(ops)):
            for wi in range(nwin):
                dl, cl, wl = lists[(oi, k, wi)]
                ne = len(dl)
                e0 = cbase * 128
                xg[e0:e0 + ne, :feat] = x_rows[cl]
                dst[e0:e0 + ne] = dl.astype(NPBF)
                we[e0:e0 + ne] = wl.astype(NPBF)
                # padded edges: dst stays 0 but we stays 0 -> sel col 0 gets 0
                cbase += int(ncw_per_op[oi][wi])
        xg = xg.reshape(ctot, 128, 128).transpose(1, 0, 2)
        xgs.append(np.ascontiguousarray(xg.reshape(128, ctot * 128)))
        dwe = np.stack([dst.reshape(ctot, 128), we.reshape(ctot, 128)],
                       axis=2)  # [C, 128, 2]
        dwes.append(np.ascontiguousarray(
            dwe.transpose(1, 0, 2).reshape(128, ctot * 2)))
    return [list(map(int, n)) for n in ncw_per_op], win_of_chunk, xgs, dwes


def _tile_rows(mat, tl=128):
    n, c = mat.shape
    nt = n // tl
    return np.ascontiguousarray(
        mat.reshape(nt, tl, c).transpose(1, 0, 2).reshape(tl, nt * c))


def _tile_w(w, pack):
    k, m = w.shape
    nb = k // 128
    t = w.reshape(nb // pack, pack, 128, m).transpose(0, 2, 1, 3)
    return np.ascontiguousarray(t.reshape((nb // pack) * 128, pack * m))


def _idx16(idx):
    return np.ascontiguousarray(
        np.tile(idx.astype(np.int16).reshape(-1, 16).T, (8, 1)))


def _wmod(W):
    W = W.astype(np.float32)
    return W[0] - W[2], W[1], 2.0 * W[2]


def _host_prep(inputs):
    d = {k: np.asarray(v) for k, v in inputs.items()}
    x = d["x"].astype(np.float32)
    l0 = np.asarray(d["l0"], np.int64)
    l1 = np.asarray(d["l1"], np.int64)
    l2 = np.asarray(d["l2"], np.int64)

    X0 = np.ascontiguousarray(x.transpose(1, 0, 2).reshape(N0, 96))
    X0b = X0.astype(NPBF)

    r0, c0, w0 = _edge_we(d["e0"], N0)
    rd0, cd0, wd0 = _sub_edges(r0, c0, w0, l0)
    rm0, cm0, wm0 = _two_hop(rd0, cd0, wd0, r0, c0, w0, N0)
    key = rm0 * N0 + cm0
    uk, inv = np.unique(key, return_inverse=True)
    wsum = np.zeros(len(uk), np.float32)
    np.add.at(wsum, inv, wm0)
    rm0, cm0, wm0 = (uk // N0).astype(np.int64), (uk % N0).astype(np.int64), wsum
    ncw_ops, win_of_chunk, xgs, dwes = _prep_l0_streams(
        [(rd0, cd0, wd0), (rm0, cm0, wm0)], X0b, N1)

    r1, c1, w1 = _edge_we(d["e1"], N1)
    rd1, cd1, wd1 = _sub_edges(r1, c1, w1, l1)
    rm1, cm1, wm1 = _two_hop(rd1, cd1, wd1, r1, c1, w1, N1)
    d1t = _tile_rows(np.ascontiguousarray(
        _dense_op(rd1, cd1, wd1, N2, N1).T).astype(NPBF))
    m1t = _tile_rows(np.ascontiguousarray(
        _dense_op(rm1, cm1, wm1, N2, N1).T).astype(NPBF))

    r2, c2, w2 = _edge_we(d["e2"], N2)
    rd2, cd2, wd2 = _sub_edges(r2, c2, w2, l2)
    rm2, cm2, wm2 = _two_hop(rd2, cd2, wd2, r2, c2, w2, N2)
    d2t = _tile_rows(np.ascontiguousarray(
        _dense_op(rd2, cd2, wd2, N3, N2).T).astype(NPBF))
    m2t = _tile_rows(np.ascontiguousarray(
        _dense_op(rm2, cm2, wm2, N3, N2).T).astype(NPBF))

    r3, c3, w3 = _edge_we(d["e3"], N3)
    S3 = _dense_op(r3, c3, w3, N3, N3)
    s3t = np.ascontiguousarray(S3.T).astype(NPBF)
    s3sqt = np.ascontiguousarray((S3 @ S3).T).astype(NPBF)

    Wm0 = _wmod(d["Wc1"])
    Wms = [_wmod(d[f"Wc{i}"]) for i in (2, 3, 4, 5)]
    eye4 = np.eye(4, dtype=np.float32)

    shared = {
        "d1t": d1t, "m1t": m1t, "d2t": d2t, "m2t": m2t,
        "s3t": s3t, "s3sqt": s3sqt,
        "x0l0t": np.ascontiguousarray(X0[l0].T).astype(NPBF),
        "l1i": _idx16(l1), "l2i": _idx16(l2),
        "epsv": np.full((128, 1), EPS, np.float32),
        "identbf": np.eye(128, dtype=np.float32).astype(NPBF),
        "iotab": np.ascontiguousarray(
            np.tile(np.arange(128, dtype=np.float32).astype(NPBF), (128, 1))),
    }
    per_core = []
    for k in range(NCORES):
        m = dict(shared)
        m["xg"] = xgs[k]
        m["dwe"] = dwes[k]
        for t in range(3):
            bw = np.zeros((96, 128), np.float32)
            for j in range(4):
                bg = 4 * k + j
                bw[3 * bg:3 * bg + 3, 32 * j:32 * j + 32] = Wm0[t]
            m[f"bigw0_{t}"] = bw.astype(NPBF)
        for lev in range(4):
            for t in range(3):
                m[f"bigw{lev + 1}_{t}"] = np.kron(eye4, Wms[lev][t]).astype(NPBF)
        for lev, nm in ((1, "b1"), (2, "b2"), (3, "b3"), (4, "b4"), (5, "b5")):
            m[f"bias{lev}"] = np.tile(d[nm].astype(np.float32), 4).reshape(128, 1)
        for li in (6, 7, 8):
            W = d[f"W{li}"].astype(np.float32)[:, 512 * k:512 * k + 512]
            m[f"w{li}"] = _tile_rows(W).astype(NPBF)  # [128, 32*512]
            m[f"g{li}"] = np.ascontiguousarray(
                d[f"g{li}"].astype(np.float32)[512 * k:512 * k + 512].reshape(4, 128).T)
            m[f"be{li}"] = np.ascontiguousarray(
                d[f"be{li}"].astype(np.float32)[512 * k:512 * k + 512].reshape(4, 128).T)
        m["w9"] = _tile_rows(
            d["W9"].astype(np.float32)[512 * k:512 * k + 512]).astype(NPBF)  # [128, 4*128]
        per_core.append(m)

    meta = {"ncw_ops": ncw_ops, "win_of_chunk": win_of_chunk}
    return per_core, meta


# ---------------------------------------------------------------- device program
def _build_nc(meta, shapes, debug=False):
    nc = bacc.Bacc("TRN2", target_bir_lowering=False, debug=False, num_devices=NCORES)
    ein = {}
    for name, arr in shapes.items():
        dt = {np.dtype(np.int16): I16, np.dtype(NPBF): BF16}.get(arr.dtype, F32)
        ein[name] = nc.dram_tensor(name, list(arr.shape), dt, kind="ExternalInput")
    out_mu = nc.dram_tensor("mu", [128, 32], F32, kind="ExternalOutput")

    tapd_loc = nc.dram_tensor("tapd_loc", [96, 512], BF16)
    tapd_all = nc.dram_tensor("tapd_all", [768, 512], BF16, addr_space="Shared")
    tapm_loc = nc.dram_tensor("tapm_loc", [96, 512], BF16)
    tapm_all = nc.dram_tensor("tapm_all", [768, 512], BF16, addr_space="Shared")
    z1n_dram = nc.dram_tensor("z1n_dram", [N1, 128], BF16)
    z2n_dram = nc.dram_tensor("z2n_dram", [N2, 128], BF16)
    x6_loc = nc.dram_tensor("x6_loc", [4, 4096], BF16)
    x6_all = nc.dram_tensor("x6_all", [32, 4096], BF16, addr_space="Shared")
    h_loc = {li: nc.dram_tensor(f"h_loc{li}", [128, 128], BF16) for li in (6, 7)}
    h_all = {li: nc.dram_tensor(f"h_all{li}", [1024, 128], BF16, addr_space="Shared")
             for li in (6, 7)}
    dbg = {}
    if debug:
        for nm, shp in (("z1T", [128, 4096]), ("tapD1", [128, 1024]),
                        ("tapM1", [128, 1024]), ("t0l1", [128, 1024]),
                        ("z2T", [128, 1024]), ("z3T", [128, 128]),
                        ("z5T", [128, 128]), ("x6T", [128, 1024]),
                        ("h6", [128, 128])):
            dbg[nm] = nc.dram_tensor("dbg_" + nm, shp, BF16, kind="ExternalOutput")
    mu_loc = nc.dram_tensor("mu_loc", [128, 32], F32)
    mu_all = nc.dram_tensor("mu_all", [128, 32], F32, addr_space="Shared")

    ncw_ops = meta["ncw_ops"]
    woc = meta["win_of_chunk"]
    ctot = len(woc)
    c_op0 = int(sum(ncw_ops[0]))
    # first/last-chunk flags per (op, window) group
    first, last = [False] * ctot, [False] * ctot
    prev = None
    for c in range(ctot):
        key = (c < c_op0, woc[c])
        if key != prev:
            first[c] = True
            if c > 0:
                last[c - 1] = True
            prev = key
    last[ctot - 1] = True

    GRP = 16

    with tile.TileContext(nc) as tc:
        with (
            tc.tile_pool(name="const", bufs=1) as cpool,
            tc.tile_pool(name="big", bufs=1) as bigpool,
            tc.tile_pool(name="work", bufs=3) as wpool,
            tc.tile_pool(name="stream", bufs=3) as spool,
            tc.tile_pool(name="wload", bufs=4) as wlpool,
            tc.tile_pool(name="mw", bufs=2) as mwpool,
            tc.tile_pool(name="psA", bufs=4, space="PSUM") as ppool,
            tc.tile_pool(name="psT", bufs=2, space="PSUM") as tpool,
        ):
            def load_const(name, dtype=BF16):
                t = cpool.tile(list(shapes[name].shape), dtype, tag=name, name=name)
                nc.sync.dma_start(out=t[:], in_=ein[name][:, :])
                return t

            eps_t = load_const("epsv", F32)
            identbf = load_const("identbf")
            identf32 = cpool.tile([128, 128], F32, tag="identf32", name="identf32")
            nc.scalar.activation(out=identf32[:], in_=identbf[:], func=AF.Copy)

            def transp(src_ap, dst_ap):
                p, f = src_ap.shape
                ps = tpool.tile([128, 128], BF16, tag="pst", name="pst")
                nc.tensor.transpose(out=ps[:f, :p], in_=src_ap, identity=identbf[:p, :p])
                nc.scalar.activation(out=dst_ap, in_=ps[:f, :p], func=AF.Copy)

            # ================= LEVEL 0: streamed selection matmuls ========
            iota_t = load_const("iotab")
            with nc.named_scope("l0_stream"):
                cur = {}
                st = dw = None
                for c in range(ctot):
                    if c % GRP == 0:
                        take = min(GRP, ctot - c)
                        st = spool.tile([128, GRP * 128], BF16, tag="xg", name="xg")
                        dw = spool.tile([128, GRP * 2], BF16, tag="dwe", name="dwe")
                        eng = nc.scalar if (c // GRP) % 2 else nc.sync
                        eng.dma_start(
                            out=st[:, :take * 128],
                            in_=ein["xg"][:, c * 128:(c + take) * 128])
                        eng.dma_start(
                            out=dw[:, :take * 2],
                            in_=ein["dwe"][:, c * 2:(c + take) * 2])
                    opi = 0 if c < c_op0 else 1
                    w = woc[c]
                    if first[c]:
                        cur[(opi, w)] = ppool.tile([128, 512], F32, tag="ps", name="ps")
                    lo = (c % GRP)
                    sel = wpool.tile([128, 128], BF16, tag="sel", name="sel", bufs=4)
                    nc.vector.tensor_scalar(
                        out=sel[:], in0=iota_t[:],
                        scalar1=dw[:, 2 * lo:2 * lo + 1],
                        scalar2=dw[:, 2 * lo + 1:2 * lo + 2],
                        op0=ALU.is_equal, op1=ALU.mult)
                    nc.tensor.matmul(
                        out=cur[(opi, w)][:, :128],
                        lhsT=st[:, lo * 128:lo * 128 + 128], rhs=sel[:],
                        start=first[c], stop=last[c])
                    if last[c]:
                        ev = wpool.tile([96, 128], BF16, tag="ev0", name="ev0", bufs=4)
                        nc.scalar.activation(out=ev[:], in_=cur[(opi, w)][:96, :128],
                                             func=AF.Copy)
                        tl = tapd_loc if opi == 0 else tapm_loc
                        nc.sync.dma_start(
                            out=tl[:, w * 128:(w + 1) * 128], in_=ev[:])
                        del cur[(opi, w)]
                        if opi == 0 and w == 3:
                            with nc.named_scope("ag_taps_d"):
                                nc.gpsimd.collective_compute(
                                    "AllGather", ALU.bypass, replica_groups=RG,
                                    ins=[tapd_loc.ap().opt()],
                                    outs=[tapd_all.ap().opt()])
            with nc.named_scope("ag_taps"):
                nc.gpsimd.collective_compute(
                    "AllGather", ALU.bypass, replica_groups=RG,
                    ins=[tapm_loc.ap().opt()], outs=[tapm_all.ap().opt()])

            # ================= LEVEL 0 einsum -> z1T ======================
            z1T = bigpool.tile([128, 4096], BF16, tag="z1T", name="z1T")
            z1n = bigpool.tile([128, 4096], BF16, tag="z1n", name="z1n")
            with nc.named_scope("l0_einsum"):
                bw0 = [load_const(f"bigw0_{t}") for t in range(3)]
                bias1 = load_const("bias1", F32)
                for j in range(8):
                    taps = []
                    t0 = wpool.tile([96, 512], BF16, tag="t0", name="t0")
                    nc.scalar.dma_start(out=t0[:], in_=ein["x0l0t"][:, 512 * j:512 * (j + 1)])
                    taps.append(t0)
                    for oi, tall in enumerate((tapd_all, tapm_all)):
                        tt = wpool.tile([96, 512], BF16, tag=f"t{oi + 1}", name=f"t{oi + 1}")
                        nc.scalar.dma_start(out=tt[:], in_=tall[96 * j:96 * j + 96, :])
                        taps.append(tt)
                    ps = ppool.tile([128, 512], F32, tag="ps", name="ps")
                    for t in range(3):
                        nc.tensor.matmul(out=ps[:, :], lhsT=bw0[t][:, :], rhs=taps[t][:],
                                         start=(t == 0), stop=(t == 2))
                    nc.scalar.activation(out=z1T[:, 512 * j:512 * (j + 1)], in_=ps[:, :],
                                         func=AF.Identity, bias=bias1[:, 0:1])
                for t in range(32):
                    transp(z1T[:, 128 * t:128 * (t + 1)], z1n[:, 128 * t:128 * (t + 1)])
                nc.sync.dma_start(
                    out=z1n_dram.ap().rearrange("(t p) f -> p t f", p=128),
                    in_=z1n[:].rearrange("p (t f) -> p t f", f=128))

            # ================= LEVEL 1: dense taps ========================
            if debug:
                nc.sync.dma_start(out=dbg["z1T"][:, :], in_=z1T[:])
            tapD1 = bigpool.tile([128, 1024], BF16, tag="tapD1", name="tapD1")
            tapM1 = bigpool.tile([128, 1024], BF16, tag="tapM1", name="tapM1")
            t0l1 = bigpool.tile([128, 1024], BF16, tag="t0l1", name="t0l1")
            with nc.named_scope("l1_taps"):
                for opi, (nm, tap) in enumerate((("d1t", tapD1), ("m1t", tapM1))):
                    psh = [ppool.tile([128, 512], F32, tag="ps", name="ps")
                           for _ in range(2)]
                    for ld in range(8):
                        stw = wlpool.tile([128, 4096], BF16, tag="wld", name="wld")
                        deng = nc.scalar if ld % 2 else nc.sync
                        deng.dma_start(out=stw[:], in_=ein[nm][:, 4096 * ld:4096 * (ld + 1)])
                        for tt in range(4):
                            t = 4 * ld + tt
                            for hh in range(2):
                                nc.tensor.matmul(
                                    out=psh[hh][:, :],
                                    lhsT=z1n[:, 128 * t:128 * (t + 1)],
                                    rhs=stw[:, 1024 * tt + 512 * hh:
                                            1024 * tt + 512 * (hh + 1)],
                                    start=(t == 0), stop=(t == 31))
                    for hh in range(2):
                        nc.scalar.activation(out=tap[:, 512 * hh:512 * (hh + 1)],
                                             in_=psh[hh][:, :], func=AF.Copy)
                l1i = load_const("l1i", I16)
                nc.gpsimd.dma_gather(
                    out_ap=t0l1[:].rearrange("p (o n) -> p o n", o=1),
                    in_ap=z1n_dram[:, :], idxs_ap=l1i[:, :],
                    num_idxs=1024, num_idxs_reg=1024, elem_size=128,
                    transpose=True, single_packet=False)

            if debug:
                nc.sync.dma_start(out=dbg["tapD1"][:, :], in_=tapD1[:])
                nc.sync.dma_start(out=dbg["tapM1"][:, :], in_=tapM1[:])
                nc.sync.dma_start(out=dbg["t0l1"][:, :], in_=t0l1[:])
            z2T = bigpool.tile([128, 1024], BF16, tag="z2T", name="z2T")
            z2n = bigpool.tile([128, 1024], BF16, tag="z2n", name="z2n")
            with nc.named_scope("l1_einsum"):
                bw1 = [load_const(f"bigw1_{t}") for t in range(3)]
                bias2 = load_const("bias2", F32)
                for w in range(2):
                    ps = ppool.tile([128, 512], F32, tag="ps", name="ps")
                    for i_, (t, tap) in enumerate(((1, tapD1), (2, tapM1), (0, t0l1))):
                        nc.tensor.matmul(out=ps[:, :], lhsT=bw1[t][:, :],
                                         rhs=tap[:, 512 * w:512 * (w + 1)],
                                         start=(i_ == 0), stop=(i_ == 2))
                    nc.scalar.activation(out=z2T[:, 512 * w:512 * (w + 1)], in_=ps[:, :],
                                         func=AF.Tanh, bias=bias2[:, 0:1])
                for t in range(8):
                    transp(z2T[:, 128 * t:128 * (t + 1)], z2n[:, 128 * t:128 * (t + 1)])
                nc.sync.dma_start(
                    out=z2n_dram.ap().rearrange("(t p) f -> p t f", p=128),
                    in_=z2n[:].rearrange("p (t f) -> p t f", f=128))

            if debug:
                nc.sync.dma_start(out=dbg["z2T"][:, :], in_=z2T[:])
            # ================= LEVEL 2 ====================================
            with nc.named_scope("l2"):
                d2c = load_const("d2t")
                m2c = load_const("m2t")
                taps2 = []
                t0l2 = wpool.tile([128, 128], BF16, tag="t0l2", name="t0l2")
                l2i = load_const("l2i", I16)
                nc.gpsimd.dma_gather(
                    out_ap=t0l2[:].rearrange("p (o n) -> p o n", o=1),
                    in_ap=z2n_dram[:, :], idxs_ap=l2i[:, :],
                    num_idxs=128, num_idxs_reg=128, elem_size=128,
                    transpose=True, single_packet=False)
                taps2.append(t0l2)
                for opi, opc in enumerate((d2c, m2c)):
                    ps = ppool.tile([128, 512], F32, tag="ps", name="ps")
                    for t in range(8):
                        nc.tensor.matmul(out=ps[:, :128],
                                         lhsT=z2n[:, 128 * t:128 * (t + 1)],
                                         rhs=opc[:, 128 * t:128 * (t + 1)],
                                         start=(t == 0), stop=(t == 7))
                    tp = wpool.tile([128, 128], BF16, tag=f"tap2{opi}", name=f"tap2{opi}")
                    nc.scalar.activation(out=tp[:], in_=ps[:, :128], func=AF.Copy)
                    taps2.append(tp)
                bw2 = [load_const(f"bigw2_{t}") for t in range(3)]
                bias3 = load_const("bias3", F32)
                ps = ppool.tile([128, 512], F32, tag="ps", name="ps")
                for t in range(3):
                    nc.tensor.matmul(out=ps[:, :128], lhsT=bw2[t][:, :],
                                     rhs=taps2[t][:, :], start=(t == 0), stop=(t == 2))
                z3T = wpool.tile([128, 128], BF16, tag="z3T", name="z3T")
                nc.scalar.activation(out=z3T[:], in_=ps[:, :128], func=AF.Tanh,
                                     bias=bias3[:, 0:1])
                if debug:
                    nc.sync.dma_start(out=dbg["z3T"][:, :], in_=z3T[:])
                z3n = wpool.tile([128, 128], BF16, tag="z3n", name="z3n")
                transp(z3T[:], z3n[:])

            # ================= LEVEL 3 ====================================
            with nc.named_scope("l3"):
                s3c = load_const("s3t")
                s3sqc = load_const("s3sqt")

                def conv3(zn, zT, bwp, bias_t, func, kp):
                    taps3 = [zT]
                    for oi, opc in enumerate((s3c, s3sqc)):
                        ps = ppool.tile([128, 512], F32, tag="ps", name="ps")
                        nc.tensor.matmul(out=ps[:, :128], lhsT=zn[:], rhs=opc[:, :],
                                         start=True, stop=True)
                        tp = wpool.tile([128, 128], BF16, tag=f"{kp}t{oi}", name=f"{kp}t{oi}")
                        nc.scalar.activation(out=tp[:], in_=ps[:, :128], func=AF.Copy)
                        taps3.append(tp)
                    bw = [load_const(f"{bwp}_{t}") for t in range(3)]
                    ps = ppool.tile([128, 512], F32, tag="ps", name="ps")
                    for t in range(3):
                        nc.tensor.matmul(out=ps[:, :128], lhsT=bw[t][:, :],
                                         rhs=taps3[t][:, :], start=(t == 0), stop=(t == 2))
                    oT = wpool.tile([128, 128], BF16, tag=f"{kp}oT", name=f"{kp}oT")
                    nc.scalar.activation(out=oT[:], in_=ps[:, :128], func=func,
                                         bias=bias_t[:, 0:1])
                    on = wpool.tile([128, 128], BF16, tag=f"{kp}on", name=f"{kp}on")
                    transp(oT[:], on[:])
                    return oT, on

                bias4 = load_const("bias4", F32)
                bias5 = load_const("bias5", F32)
                z4T, z4n = conv3(z3n[:], z3T[:], "bigw3", bias4, AF.Tanh, "c4")
                z5T, _ = conv3(z4n[:], z4T[:], "bigw4", bias5, AF.Identity, "c5")

            if debug:
                nc.sync.dma_start(out=dbg["z5T"][:, :], in_=z5T[:])
            # ================= MLP input assembly =========================
            with nc.named_scope("mlp_in"):
                z5n = wpool.tile([128, 128], BF16, tag="z5n", name="z5n")
                transp(z5T[:], z5n[:])
                for b in range(4):
                    nc.sync.dma_start(
                        out=x6_loc.ap()[b:b + 1, :].rearrange("o (n h) -> n (o h)", h=32),
                        in_=z5n[:, 32 * b:32 * b + 32])
                nc.gpsimd.collective_compute(
                    "AllGather", ALU.bypass, replica_groups=RG,
                    ins=[x6_loc.ap().opt()], outs=[x6_all.ap().opt()])

            # ================= MLP ========================================
            def mlp_layer(nm, src_sb, out_sb):
                g_t = load_const("g" + nm[1], F32)
                be_t = load_const("be" + nm[1], F32)
                wts = []
                for i in range(4):
                    wt = mwpool.tile([128, 4096], BF16, tag=f"mw{i}", name=f"mw{i}")
                    nc.scalar.dma_start(out=wt[:], in_=ein[nm][:, 4096 * i:4096 * (i + 1)])
                    wts.append(wt)
                ps = ppool.tile([128, 512], F32, tag="ps", name="ps")
                for kc in range(32):
                    nc.tensor.matmul(
                        out=ps[:32, :],
                        lhsT=src_sb[:, 32 * kc:32 * kc + 32],
                        rhs=wts[kc // 8][:, 512 * (kc % 8):512 * (kc % 8 + 1)],
                        start=(kc == 0), stop=(kc == 31))
                hb = wpool.tile([32, 512], F32, tag="hb", name="hb")
                nc.scalar.activation(out=hb[:], in_=ps[:32, :], func=AF.Copy)
                for mm in range(4):
                    pst = ppool.tile([128, 512], F32, tag="ps", name="ps")
                    nc.tensor.transpose(out=pst[:128, :32],
                                        in_=hb[:, 128 * mm:128 * (mm + 1)],
                                        identity=identf32[:32, :32])
                    t = wpool.tile([128, 32], F32, tag="b_t", name="b_t")
                    nc.vector.tensor_copy(t[:], pst[:128, :32])
                    s1 = wpool.tile([128, 1], F32, tag="b_s1", name="b_s1")
                    nc.vector.tensor_reduce(out=s1[:], in_=t[:], axis=AX.X, op=ALU.add)
                    mu_ = wpool.tile([128, 1], F32, tag="b_mu", name="b_mu")
                    nc.vector.tensor_scalar_mul(mu_[:], s1[:], 1.0 / 32.0)
                    sq = wpool.tile([128, 32], F32, tag="b_sq", name="b_sq")
                    nc.vector.tensor_mul(sq[:], t[:], t[:])
                    s2_ = wpool.tile([128, 1], F32, tag="b_s2", name="b_s2")
                    nc.vector.tensor_reduce(out=s2_[:], in_=sq[:], axis=AX.X, op=ALU.add)
                    var = wpool.tile([128, 1], F32, tag="b_var", name="b_var")
                    nc.vector.scalar_tensor_tensor(out=var[:], in0=mu_[:], scalar=-1.0,
                                                   in1=mu_[:], op0=ALU.mult, op1=ALU.mult)
                    nc.vector.scalar_tensor_tensor(out=var[:], in0=s2_[:], scalar=1.0 / 32.0,
                                                   in1=var[:], op0=ALU.mult, op1=ALU.add)
                    sd = wpool.tile([128, 1], F32, tag="b_sd", name="b_sd")
                    nc.scalar.activation(out=sd[:], in_=var[:], func=AF.Sqrt,
                                         bias=eps_t[:, 0:1])
                    rs = wpool.tile([128, 1], F32, tag="b_rs", name="b_rs")
                    nc.vector.reciprocal(rs[:], sd[:])
                    a_ = wpool.tile([128, 1], F32, tag="b_a", name="b_a")
                    nc.vector.tensor_mul(a_[:], rs[:], g_t[:, mm:mm + 1])
                    sh = wpool.tile([128, 1], F32, tag="b_sh", name="b_sh")
                    nc.vector.scalar_tensor_tensor(out=sh[:], in0=mu_[:], scalar=-1.0,
                                                   in1=a_[:], op0=ALU.mult, op1=ALU.mult)
                    nc.vector.tensor_add(sh[:], sh[:], be_t[:, mm:mm + 1])
                    nc.scalar.activation(out=out_sb[:, 32 * mm:32 * mm + 32], in_=t[:],
                                         func=AF.Relu, scale=a_[:, 0:1], bias=sh[:, 0:1])

            x6T = bigpool.tile([128, 1024], BF16, tag="x6T", name="x6T")
            with nc.named_scope("mlp6"):
                xbm = wpool.tile([32, 4096], BF16, tag="xbm", name="xbm")
                nc.sync.dma_start(out=xbm[:], in_=x6_all[:, :])
                for t in range(32):
                    ps = tpool.tile([128, 128], BF16, tag="pst", name="pst")
                    nc.tensor.transpose(out=ps[:128, :32],
                                        in_=xbm[:, 128 * t:128 * (t + 1)],
                                        identity=identbf[:32, :32])
                    nc.vector.tensor_copy(x6T[:, 32 * t:32 * t + 32], ps[:128, :32])
                if debug:
                    nc.sync.dma_start(out=dbg["x6T"][:, :], in_=x6T[:])
                h6 = bigpool.tile([128, 128], BF16, tag="h6", name="h6")
                mlp_layer("w6", x6T, h6)
                if debug:
                    nc.sync.dma_start(out=dbg["h6"][:, :], in_=h6[:])
                nc.sync.dma_start(out=h_loc[6][:, :], in_=h6[:])
                nc.gpsimd.collective_compute(
                    "AllGather", ALU.bypass, replica_groups=RG,
                    ins=[h_loc[6].ap().opt()], outs=[h_all[6].ap().opt()])
            with nc.named_scope("mlp7"):
                x7T = bigpool.tile([128, 1024], BF16, tag="x7T", name="x7T")
                nc.sync.dma_start(out=x7T[:].rearrange("p (j c) -> p j c", c=128),
                                  in_=h_all[6][:, :].rearrange("(j p) c -> p j c", p=128))
                h7 = bigpool.tile([128, 128], BF16, tag="h7", name="h7")
                mlp_layer("w7", x7T, h7)
                nc.sync.dma_start(out=h_loc[7][:, :], in_=h7[:])
                nc.gpsimd.collective_compute(
                    "AllGather", ALU.bypass, replica_groups=RG,
                    ins=[h_loc[7].ap().opt()], outs=[h_all[7].ap().opt()])
            with nc.named_scope("mlp8"):
                x8T = bigpool.tile([128, 1024], BF16, tag="x8T", name="x8T")
                nc.sync.dma_start(out=x8T[:].rearrange("p (j c) -> p j c", c=128),
                                  in_=h_all[7][:, :].rearrange("(j p) c -> p j c", p=128))
                h8 = bigpool.tile([128, 128], BF16, tag="h8", name="h8")
                mlp_layer("w8", x8T, h8)

            with nc.named_scope("mlp9"):
                w9t = load_const("w9")
                ps9 = ppool.tile([128, 512], F32, tag="ps", name="ps")
                for kc in range(4):
                    nc.tensor.matmul(out=ps9[:32, :128],
                                     lhsT=h8[:, 32 * kc:32 * kc + 32],
                                     rhs=w9t[:, kc * 128:(kc + 1) * 128],
                                     start=(kc == 0), stop=(kc == 3))
                mub = wpool.tile([32, 128], F32, tag="mub", name="mub")
                nc.scalar.activation(out=mub[:], in_=ps9[:32, :128], func=AF.Copy)
                ps9t = ppool.tile([128, 512], F32, tag="ps", name="ps")
                nc.tensor.transpose(out=ps9t[:128, :32], in_=mub[:],
                                    identity=identf32[:32, :32])
                mu_sb = wpool.tile([128, 32], F32, tag="mu_sb", name="mu_sb")
                nc.scalar.activation(out=mu_sb[:], in_=ps9t[:128, :32], func=AF.Copy)
                nc.sync.dma_start(out=mu_loc[:, :], in_=mu_sb[:])
                nc.gpsimd.collective_compute(
                    "AllReduce", ALU.add, replica_groups=RG,
                    ins=[mu_loc.ap().opt()], outs=[mu_all.ap().opt()])
                tot = wpool.tile([128, 32], F32, tag="f_tot", name="f_tot")
                nc.sync.dma_start(out=tot[:], in_=mu_all[0:128, :])
                s1 = wpool.tile([128, 1], F32, tag="f_s1", name="f_s1")
                nc.vector.tensor_reduce(out=s1[:], in_=tot[:], axis=AX.X, op=ALU.add)
                mu_ = wpool.tile([128, 1], F32, tag="f_mu", name="f_mu")
                nc.vector.tensor_scalar_mul(mu_[:], s1[:], 1.0 / 32.0)
                sq = wpool.tile([128, 32], F32, tag="f_sq", name="f_sq")
                nc.vector.tensor_mul(sq[:], tot[:], tot[:])
                s2_ = wpool.tile([128, 1], F32, tag="f_s2", name="f_s2")
                nc.vector.tensor_reduce(out=s2_[:], in_=sq[:], axis=AX.X, op=ALU.add)
                var = wpool.tile([128, 1], F32, tag="f_var", name="f_var")
                nc.vector.scalar_tensor_tensor(out=var[:], in0=mu_[:], scalar=-1.0,
                                               in1=mu_[:], op0=ALU.mult, op1=ALU.mult)
                nc.vector.scalar_tensor_tensor(out=var[:], in0=s2_[:], scalar=1.0 / 32.0,
                                               in1=var[:], op0=ALU.mult, op1=ALU.add)
                sdf = wpool.tile([128, 1], F32, tag="f_sd", name="f_sd")
                nc.scalar.activation(out=sdf[:], in_=var[:], func=AF.Sqrt,
                                     bias=eps_t[:, 0:1])
                rs = wpool.tile([128, 1], F32, tag="f_rs", name="f_rs")
                nc.vector.reciprocal(rs[:], sdf[:])
                neg = wpool.tile([128, 1], F32, tag="f_neg", name="f_neg")
                nc.vector.scalar_tensor_tensor(out=neg[:], in0=mu_[:], scalar=-1.0,
                                               in1=rs[:], op0=ALU.mult, op1=ALU.mult)
                outt = wpool.tile([128, 32], F32, tag="f_out", name="f_out")
                nc.scalar.activation(out=outt[:], in_=tot[:], func=AF.Identity,
                                     scale=rs[:, 0:1], bias=neg[:, 0:1])
                nc.sync.dma_start(out=out_mu[:, :], in_=outt[:])

    nc.compile()
    return nc


# ---------------------------------------------------------------- entry point
def kernel(**inputs) -> np.ndarray:
    per_core, meta = _host_prep(inputs)
    key = (len(meta["win_of_chunk"]), tuple(meta["win_of_chunk"]),
           tuple(tuple(n) for n in meta["ncw_ops"]))
    if _CACHE.get("key") != key:
        _CACHE["prog"] = _build_nc(meta, per_core[0])
        _CACHE["key"] = key
    nc = _CACHE["prog"]
    res = bass_utils.run_bass_kernel_spmd(nc, per_core, core_ids=list(range(NCORES)))
    return np.ascontiguousarray(res.results[0]["mu"].T)


# revision 16
# speedup vs baseline: 1.0922x; 1.0922x over previous
"""Trainium2 Bass kernel for nn_Encoder_base (5x ChebConv GNN + pool + MLP).

Restructured for speed:
  - Only pooled rows of each ChebConv are needed downstream, so each level's
    Chebyshev taps are computed directly at the pooled nodes via
    host-precomputed operators: tap1 = S[pool,:] z, tap2 = (S^2)[pool,:] z.
  - Level 0 applies them as per-edge selection matmuls whose source rows are
    HOST-gathered from x (x is a kernel input), streamed with the weighted
    selection matrices as one bf16 stream -> zero device gathers, zero DVE.
  - Levels 1-2 apply the pooled operators as dense bf16 matmuls; level 3
    (no pooling) uses dense S3 / S3^2.
  - Everything on the matmul path is bf16 (fp32 matmul is 4 cycles/row on
    TRN2, bf16 is 1 and gets fast weight load).
  - Distribution: level 0 edge-sharded by pooled dest (one small AllGather
    of the taps); levels 1-3 batch-sharded (4 batches/core, replicated
    small dense ops); MLP output-feature sharded with per-layer AllGathers.
"""
import numpy as np
import ml_dtypes
import concourse.bass as bass
import concourse.bacc as bacc
import concourse.tile as tile
from concourse import mybir, bass_utils

F32 = mybir.dt.float32
BF16 = mybir.dt.bfloat16
I16 = mybir.dt.int16
NPBF = ml_dtypes.bfloat16
AF = mybir.ActivationFunctionType
ALU = mybir.AluOpType
AX = mybir.AxisListType
RG = [list(range(8))]
NCORES = 8
N0, N1, N2, N3 = 16384, 4096, 1024, 128
EPS = 1e-5

_CACHE = {}


# ---------------------------------------------------------------- host prep
def _edge_we(e, n):
    row = np.asarray(e[0], np.int64)
    col = np.asarray(e[1], np.int64)
    deg = np.bincount(row, minlength=n).astype(np.float32)
    dis = np.where(deg > 0, 1.0 / np.sqrt(np.maximum(deg, 1.0)), 0.0).astype(np.float32)
    return row, col, -(dis[row] * dis[col]).astype(np.float32)


def _sort_by_row(row, col, we):
    order = np.argsort(row, kind="stable")
    return row[order], col[order], we[order]


def _sub_edges(row, col, we, pool_idx):
    row, col, we = _sort_by_row(row, col, we)
    starts = np.searchsorted(row, pool_idx, side="left")
    ends = np.searchsorted(row, pool_idx, side="right")
    cnt = ends - starts
    tot = int(cnt.sum())
    pos = np.repeat(starts, cnt) + (np.arange(tot) - np.repeat(np.cumsum(cnt) - cnt, cnt))
    nr = np.repeat(np.arange(len(pool_idx), dtype=np.int64), cnt)
    return nr, col[pos], we[pos]


def _two_hop(rd, cd, wd, row, col, we, n):
    row, col, we = _sort_by_row(row, col, we)
    indptr = np.searchsorted(row, np.arange(n + 1))
    s, e = indptr[cd], indptr[cd + 1]
    cnt = e - s
    tot = int(cnt.sum())
    pos = np.repeat(s, cnt) + (np.arange(tot) - np.repeat(np.cumsum(cnt) - cnt, cnt))
    return np.repeat(rd, cnt), col[pos], np.repeat(wd, cnt) * we[pos]


def _dense_op(rd, cd, wd, n_rows, n_cols):
    m = np.zeros((n_rows, n_cols), np.float32)
    np.add.at(m, (rd, cd), wd)
    return m


def _prep_l0_streams(ops, x_rows, n_dest, window=128):
    """Per-edge (xg | sel) bf16 streams, dest-sharded over cores."""
    per = n_dest // NCORES
    nwin = per // window
    feat = x_rows.shape[1]
    ncw_per_op, lists = [], {}
    for oi, (rd, cd, wd) in enumerate(ops):
        rd, cd, wd = _sort_by_row(rd, cd, wd)
        counts = np.zeros((NCORES, nwin), np.int64)
        for k in range(NCORES):
            lo = k * per
            for wi in range(nwin):
                a = np.searchsorted(rd, lo + wi * window, side="left")
                b = np.searchsorted(rd, lo + (wi + 1) * window, side="left")
                lists[(oi, k, wi)] = (rd[a:b] - (lo + wi * window), cd[a:b], wd[a:b])
                counts[k, wi] = (b - a + 127) // 128
        ncw_per_op.append(np.maximum(counts.max(axis=0), 1).astype(np.int64))
    ctot = int(sum(n.sum() for n in ncw_per_op))
    win_of_chunk = []
    for oi in range(len(ops)):
        for wi in range(nwin):
            win_of_chunk += [wi] * int(ncw_per_op[oi][wi])
    xgs, dwes = [], []
    for k in range(NCORES):
        xg = np.zeros((ctot * 128, 128), NPBF)
        dst = np.zeros((ctot * 128,), np.float32)
        we = np.zeros((ctot * 128,), np.float32)
        cbase = 0
        for oi in range(len(ops)):
            for wi in range(nwin):
                dl, cl, wl = lists[(oi, k, wi)]
                ne = len(dl)
                e0 = cbase * 128
                xg[e0:e0 + ne, :feat] = x_rows[cl]
                dst[e0:e0 + ne] = dl
                we[e0:e0 + ne] = wl
                # padded edges: dst stays 0 but we stays 0 -> sel col 0 gets 0
                cbase += int(ncw_per_op[oi][wi])
        xg = xg.reshape(ctot, 128, 128).transpose(1, 0, 2)
        xgs.append(np.ascontiguousarray(xg.reshape(128, ctot * 128)))
        dwe = np.stack([dst.reshape(ctot, 128), we.reshape(ctot, 128)],
                       axis=2)  # [C, 128, 2]
        dwes.append(np.ascontiguousarray(
            dwe.transpose(1, 0, 2).reshape(128, ctot * 2)))
    return [list(map(int, n)) for n in ncw_per_op], win_of_chunk, xgs, dwes


def _tile_rows(mat, tl=128):
    n, c = mat.shape
    nt = n // tl
    return np.ascontiguousarray(
        mat.reshape(nt, tl, c).transpose(1, 0, 2).reshape(tl, nt * c))


def _tile_w(w, pack):
    k, m = w.shape
    nb = k // 128
    t = w.reshape(nb // pack, pack, 128, m).transpose(0, 2, 1, 3)
    return np.ascontiguousarray(t.reshape((nb // pack) * 128, pack * m))


def _idx16(idx):
    return np.ascontiguousarray(
        np.tile(idx.astype(np.int16).reshape(-1, 16).T, (8, 1)))


def _wmod(W):
    W = W.astype(np.float32)
    return W[0] - W[2], W[1], 2.0 * W[2]


def _host_prep(inputs):
    d = {k: np.asarray(v) for k, v in inputs.items()}
    x = d["x"].astype(np.float32)
    l0 = np.asarray(d["l0"], np.int64)
    l1 = np.asarray(d["l1"], np.int64)
    l2 = np.asarray(d["l2"], np.int64)

    X0 = np.ascontiguousarray(x.transpose(1, 0, 2).reshape(N0, 96))
    X0b = X0.astype(NPBF)

    r0, c0, w0 = _edge_we(d["e0"], N0)
    rd0, cd0, wd0 = _sub_edges(r0, c0, w0, l0)
    rm0, cm0, wm0 = _two_hop(rd0, cd0, wd0, r0, c0, w0, N0)
    key = rm0 * N0 + cm0
    uk, inv = np.unique(key, return_inverse=True)
    wsum = np.zeros(len(uk), np.float32)
    np.add.at(wsum, inv, wm0)
    rm0, cm0, wm0 = (uk // N0).astype(np.int64), (uk % N0).astype(np.int64), wsum
    ncw_ops, win_of_chunk, xgs, dwes = _prep_l0_streams(
        [(rd0, cd0, wd0), (rm0, cm0, wm0)], X0b, N1)

    r1, c1, w1 = _edge_we(d["e1"], N1)
    rd1, cd1, wd1 = _sub_edges(r1, c1, w1, l1)
    rm1, cm1, wm1 = _two_hop(rd1, cd1, wd1, r1, c1, w1, N1)
    d1t = _tile_rows(np.ascontiguousarray(
        _dense_op(rd1, cd1, wd1, N2, N1).T).astype(NPBF))
    m1t = _tile_rows(np.ascontiguousarray(
        _dense_op(rm1, cm1, wm1, N2, N1).T).astype(NPBF))

    r2, c2, w2 = _edge_we(d["e2"], N2)
    rd2, cd2, wd2 = _sub_edges(r2, c2, w2, l2)
    rm2, cm2, wm2 = _two_hop(rd2, cd2, wd2, r2, c2, w2, N2)
    d2t = _tile_rows(np.ascontiguousarray(
        _dense_op(rd2, cd2, wd2, N3, N2).T).astype(NPBF))
    m2t = _tile_rows(np.ascontiguousarray(
        _dense_op(rm2, cm2, wm2, N3, N2).T).astype(NPBF))

    r3, c3, w3 = _edge_we(d["e3"], N3)
    S3 = _dense_op(r3, c3, w3, N3, N3)
    s3t = np.ascontiguousarray(S3.T).astype(NPBF)
    s3sqt = np.ascontiguousarray((S3 @ S3).T).astype(NPBF)

    Wm0 = _wmod(d["Wc1"])
    Wms = [_wmod(d[f"Wc{i}"]) for i in (2, 3, 4, 5)]
    eye4 = np.eye(4, dtype=np.float32)

    shared = {
        "d1t": d1t, "m1t": m1t, "d2t": d2t, "m2t": m2t,
        "s3t": s3t, "s3sqt": s3sqt,
        "x0l0t": np.ascontiguousarray(X0[l0].T).astype(NPBF),
        "l1i": _idx16(l1), "l2i": _idx16(l2),
        "epsv": np.full((128, 1), EPS, np.float32),
        "identbf": np.eye(128, dtype=np.float32).astype(NPBF),
        "iotab": np.ascontiguousarray(
            np.tile(np.arange(128, dtype=np.float32).astype(NPBF), (128, 1))),
    }
    per_core = []
    for k in range(NCORES):
        m = dict(shared)
        m["xg"] = xgs[k]
        m["dwe"] = dwes[k]
        for t in range(3):
            bw = np.zeros((96, 128), np.float32)
            for j in range(4):
                bg = 4 * k + j
                bw[3 * bg:3 * bg + 3, 32 * j:32 * j + 32] = Wm0[t]
            m[f"bigw0_{t}"] = bw.astype(NPBF)
        for lev in range(4):
            for t in range(3):
                m[f"bigw{lev + 1}_{t}"] = np.kron(eye4, Wms[lev][t]).astype(NPBF)
        for lev, nm in ((1, "b1"), (2, "b2"), (3, "b3"), (4, "b4"), (5, "b5")):
            m[f"bias{lev}"] = np.tile(d[nm].astype(np.float32), 4).reshape(128, 1)
        for li in (6, 7, 8):
            W = d[f"W{li}"].astype(np.float32)[:, 512 * k:512 * k + 512]
            m[f"w{li}"] = _tile_rows(W).astype(NPBF)  # [128, 32*512]
            m[f"g{li}"] = np.ascontiguousarray(
                d[f"g{li}"].astype(np.float32)[512 * k:512 * k + 512].reshape(4, 128).T)
            m[f"be{li}"] = np.ascontiguousarray(
                d[f"be{li}"].astype(np.float32)[512 * k:512 * k + 512].reshape(4, 128).T)
        m["w9"] = _tile_rows(
            d["W9"].astype(np.float32)[512 * k:512 * k + 512]).astype(NPBF)  # [128, 4*128]
        per_core.append(m)

    meta = {"ncw_ops": ncw_ops, "win_of_chunk": win_of_chunk}
    return per_core, meta


# ---------------------------------------------------------------- device program
def _build_nc(meta, shapes, debug=False):
    nc = bacc.Bacc("TRN2", target_bir_lowering=False, debug=False, num_devices=NCORES)
    ein = {}
    for name, arr in shapes.items():
        dt = {np.dtype(np.int16): I16, np.dtype(NPBF): BF16}.get(arr.dtype, F32)
        ein[name] = nc.dram_tensor(name, list(arr.shape), dt, kind="ExternalInput")
    out_mu = nc.dram_tensor("mu", [128, 32], F32, kind="ExternalOutput")

    tapd_loc = nc.dram_tensor("tapd_loc", [96, 512], BF16)
    tapd_all = nc.dram_tensor("tapd_all", [768, 512], BF16, addr_space="Shared")
    tapm_loc = nc.dram_tensor("tapm_loc", [96, 512], BF16)
    tapm_all = nc.dram_tensor("tapm_all", [768, 512], BF16, addr_space="Shared")
    z1n_dram = nc.dram_tensor("z1n_dram", [N1, 128], BF16)
    z2n_dram = nc.dram_tensor("z2n_dram", [N2, 128], BF16)
    x6_loc = nc.dram_tensor("x6_loc", [4, 4096], BF16)
    x6_all = nc.dram_tensor("x6_all", [32, 4096], BF16, addr_space="Shared")
    h_loc = {li: nc.dram_tensor(f"h_loc{li}", [128, 128], BF16) for li in (6, 7)}
    h_all = {li: nc.dram_tensor(f"h_all{li}", [1024, 128], BF16, addr_space="Shared")
             for li in (6, 7)}
    dbg = {}
    if debug:
        for nm, shp in (("z1T", [128, 4096]), ("tapD1", [128, 1024]),
                        ("tapM1", [128, 1024]), ("t0l1", [128, 1024]),
                        ("z2T", [128, 1024]), ("z3T", [128, 128]),
                        ("z5T", [128, 128]), ("x6T", [128, 1024]),
                        ("h6", [128, 128])):
            dbg[nm] = nc.dram_tensor("dbg_" + nm, shp, BF16, kind="ExternalOutput")
    mu_loc = nc.dram_tensor("mu_loc", [128, 32], F32)
    mu_all = nc.dram_tensor("mu_all", [128, 32], F32, addr_space="Shared")

    ncw_ops = meta["ncw_ops"]
    woc = meta["win_of_chunk"]
    ctot = len(woc)
    c_op0 = int(sum(ncw_ops[0]))
    # first/last-chunk flags per (op, window) group
    first, last = [False] * ctot, [False] * ctot
    prev = None
    for c in range(ctot):
        key = (c < c_op0, woc[c])
        if key != prev:
            first[c] = True
            if c > 0:
                last[c - 1] = True
            prev = key
    last[ctot - 1] = True

    GRP = 16

    with tile.TileContext(nc) as tc:
        with (
            tc.tile_pool(name="const", bufs=1) as cpool,
            tc.tile_pool(name="big", bufs=1) as bigpool,
            tc.tile_pool(name="work", bufs=3) as wpool,
            tc.tile_pool(name="stream", bufs=3) as spool,
            tc.tile_pool(name="wload", bufs=3) as wlpool,
            tc.tile_pool(name="mw", bufs=2) as mwpool,
            tc.tile_pool(name="psA", bufs=4, space="PSUM") as ppool,
            tc.tile_pool(name="psT", bufs=2, space="PSUM") as tpool,
        ):
            def load_const(name, dtype=BF16):
                t = cpool.tile(list(shapes[name].shape), dtype, tag=name, name=name)
                nc.sync.dma_start(out=t[:], in_=ein[name][:, :])
                return t

            eps_t = load_const("epsv", F32)
            identbf = load_const("identbf")
            identf32 = cpool.tile([128, 128], F32, tag="identf32", name="identf32")
            nc.scalar.activation(out=identf32[:], in_=identbf[:], func=AF.Copy)

            def transp(src_ap, dst_ap):
                p, f = src_ap.shape
                ps = tpool.tile([128, 128], BF16, tag="pst", name="pst")
                nc.tensor.transpose(out=ps[:f, :p], in_=src_ap, identity=identbf[:p, :p])
                nc.scalar.activation(out=dst_ap, in_=ps[:f, :p], func=AF.Copy)

            # ================= LEVEL 0: streamed selection matmuls ========
            iota_t = load_const("iotab")
            with nc.named_scope("l0_stream"):
                cur = {}
                st = dw = None
                for c in range(ctot):
                    if c % GRP == 0:
                        take = min(GRP, ctot - c)
                        st = spool.tile([128, GRP * 128], BF16, tag="xg", name="xg")
                        dw = spool.tile([128, GRP * 2], F32, tag="dwe", name="dwe")
                        eng = nc.scalar if (c // GRP) % 2 else nc.sync
                        eng.dma_start(
                            out=st[:, :take * 128],
                            in_=ein["xg"][:, c * 128:(c + take) * 128])
                        eng.dma_start(
                            out=dw[:, :take * 2],
                            in_=ein["dwe"][:, c * 2:(c + take) * 2])
                    opi = 0 if c < c_op0 else 1
                    w = woc[c]
                    if first[c]:
                        cur[(opi, w)] = ppool.tile([128, 512], F32, tag="ps", name="ps")
                    lo = (c % GRP)
                    sel = wpool.tile([128, 128], BF16, tag="sel", name="sel", bufs=4)
                    nc.vector.tensor_scalar(
                        out=sel[:], in0=iota_t[:],
                        scalar1=dw[:, 2 * lo:2 * lo + 1],
                        scalar2=dw[:, 2 * lo + 1:2 * lo + 2],
                        op0=ALU.is_equal, op1=ALU.mult)
                    nc.tensor.matmul(
                        out=cur[(opi, w)][:, :128],
                        lhsT=st[:, lo * 128:lo * 128 + 128], rhs=sel[:],
                        start=first[c], stop=last[c])
                    if last[c]:
                        ev = wpool.tile([96, 128], BF16, tag="ev0", name="ev0", bufs=4)
                        nc.scalar.activation(out=ev[:], in_=cur[(opi, w)][:96, :128],
                                             func=AF.Copy)
                        tl = tapd_loc if opi == 0 else tapm_loc
                        nc.sync.dma_start(
                            out=tl[:, w * 128:(w + 1) * 128], in_=ev[:])
                        del cur[(opi, w)]
                        if opi == 0 and w == 3:
                            with nc.named_scope("ag_taps_d"):
                                nc.gpsimd.collective_compute(
                                    "AllGather", ALU.bypass, replica_groups=RG,
                                    ins=[tapd_loc.ap().opt()],
                                    outs=[tapd_all.ap().opt()])
            with nc.named_scope("ag_taps"):
                nc.gpsimd.collective_compute(
                    "AllGather", ALU.bypass, replica_groups=RG,
                    ins=[tapm_loc.ap().opt()], outs=[tapm_all.ap().opt()])

            # ================= LEVEL 0 einsum -> z1T ======================
            z1T = bigpool.tile([128, 4096], BF16, tag="z1T", name="z1T")
            z1n = bigpool.tile([128, 4096], BF16, tag="z1n", name="z1n")
            with nc.named_scope("l0_einsum"):
                bw0 = [load_const(f"bigw0_{t}") for t in range(3)]
                bias1 = load_const("bias1", F32)
                for j in range(8):
                    taps = []
                    t0 = wpool.tile([96, 512], BF16, tag="t0", name="t0")
                    nc.scalar.dma_start(out=t0[:], in_=ein["x0l0t"][:, 512 * j:512 * (j + 1)])
                    taps.append(t0)
                    for oi, tall in enumerate((tapd_all, tapm_all)):
                        tt = wpool.tile([96, 512], BF16, tag=f"t{oi + 1}", name=f"t{oi + 1}")
                        nc.scalar.dma_start(out=tt[:], in_=tall[96 * j:96 * j + 96, :])
                        taps.append(tt)
                    ps = ppool.tile([128, 512], F32, tag="ps", name="ps")
                    for t in range(3):
                        nc.tensor.matmul(out=ps[:, :], lhsT=bw0[t][:, :], rhs=taps[t][:],
                                         start=(t == 0), stop=(t == 2))
                    nc.scalar.activation(out=z1T[:, 512 * j:512 * (j + 1)], in_=ps[:, :],
                                         func=AF.Identity, bias=bias1[:, 0:1])
                for t in range(32):
                    transp(z1T[:, 128 * t:128 * (t + 1)], z1n[:, 128 * t:128 * (t + 1)])
                nc.sync.dma_start(
                    out=z1n_dram.ap().rearrange("(t p) f -> p t f", p=128),
                    in_=z1n[:].rearrange("p (t f) -> p t f", f=128))

            # ================= LEVEL 1: dense taps ========================
            if debug:
                nc.sync.dma_start(out=dbg["z1T"][:, :], in_=z1T[:])
            tapD1 = bigpool.tile([128, 1024], BF16, tag="tapD1", name="tapD1")
            tapM1 = bigpool.tile([128, 1024], BF16, tag="tapM1", name="tapM1")
            t0l1 = bigpool.tile([128, 1024], BF16, tag="t0l1", name="t0l1")
            with nc.named_scope("l1_taps"):
                for opi, (nm, tap) in enumerate((("d1t", tapD1), ("m1t", tapM1))):
                    psh = [ppool.tile([128, 512], F32, tag="ps", name="ps")
                           for _ in range(2)]
                    for ld in range(8):
                        stw = wlpool.tile([128, 4096], BF16, tag="wld", name="wld")
                        deng = nc.scalar if ld % 2 else nc.sync
                        deng.dma_start(out=stw[:], in_=ein[nm][:, 4096 * ld:4096 * (ld + 1)])
                        for tt in range(4):
                            t = 4 * ld + tt
                            for hh in range(2):
                                nc.tensor.matmul(
                                    out=psh[hh][:, :],
                                    lhsT=z1n[:, 128 * t:128 * (t + 1)],
                                    rhs=stw[:, 1024 * tt + 512 * hh:
                                            1024 * tt + 512 * (hh + 1)],
                                    start=(t == 0), stop=(t == 31))
                    for hh in range(2):
                        nc.scalar.activation(out=tap[:, 512 * hh:512 * (hh + 1)],
                                             in_=psh[hh][:, :], func=AF.Copy)
                l1i = load_const("l1i", I16)
                nc.gpsimd.dma_gather(
                    out_ap=t0l1[:].rearrange("p (o n) -> p o n", o=1),
                    in_ap=z1n_dram[:, :], idxs_ap=l1i[:, :],
                    num_idxs=1024, num_idxs_reg=1024, elem_size=128,
                    transpose=True, single_packet=False)

            if debug:
                nc.sync.dma_start(out=dbg["tapD1"][:, :], in_=tapD1[:])
                nc.sync.dma_start(out=dbg["tapM1"][:, :], in_=tapM1[:])
                nc.sync.dma_start(out=dbg["t0l1"][:, :], in_=t0l1[:])
            z2T = bigpool.tile([128, 1024], BF16, tag="z2T", name="z2T")
            z2n = bigpool.tile([128, 1024], BF16, tag="z2n", name="z2n")
            with nc.named_scope("l1_einsum"):
                bw1 = [load_const(f"bigw1_{t}") for t in range(3)]
                bias2 = load_const("bias2", F32)
                for w in range(2):
                    ps = ppool.tile([128, 512], F32, tag="ps", name="ps")
                    for i_, (t, tap) in enumerate(((1, tapD1), (2, tapM1), (0, t0l1))):
                        nc.tensor.matmul(out=ps[:, :], lhsT=bw1[t][:, :],
                                         rhs=tap[:, 512 * w:512 * (w + 1)],
                                         start=(i_ == 0), stop=(i_ == 2))
                    nc.scalar.activation(out=z2T[:, 512 * w:512 * (w + 1)], in_=ps[:, :],
                                         func=AF.Tanh, bias=bias2[:, 0:1])
                for t in range(8):
                    transp(z2T[:, 128 * t:128 * (t + 1)], z2n[:, 128 * t:128 * (t + 1)])
                nc.sync.dma_start(
                    out=z2n_dram.ap().rearrange("(t p) f -> p t f", p=128),
                    in_=z2n[:].rearrange("p (t f) -> p t f", f=128))

            if debug:
                nc.sync.dma_start(out=dbg["z2T"][:, :], in_=z2T[:])
            # ================= LEVEL 2 ====================================
            with nc.named_scope("l2"):
                d2c = load_const("d2t")
                m2c = load_const("m2t")
                taps2 = []
                t0l2 = wpool.tile([128, 128], BF16, tag="t0l2", name="t0l2")
                l2i = load_const("l2i", I16)
                nc.gpsimd.dma_gather(
                    out_ap=t0l2[:].rearrange("p (o n) -> p o n", o=1),
                    in_ap=z2n_dram[:, :], idxs_ap=l2i[:, :],
                    num_idxs=128, num_idxs_reg=128, elem_size=128,
                    transpose=True, single_packet=False)
                taps2.append(t0l2)
                for opi, opc in enumerate((d2c, m2c)):
                    ps = ppool.tile([128, 512], F32, tag="ps", name="ps")
                    for t in range(8):
                        nc.tensor.matmul(out=ps[:, :128],
                                         lhsT=z2n[:, 128 * t:128 * (t + 1)],
                                         rhs=opc[:, 128 * t:128 * (t + 1)],
                                         start=(t == 0), stop=(t == 7))
                    tp = wpool.tile([128, 128], BF16, tag=f"tap2{opi}", name=f"tap2{opi}")
                    nc.scalar.activation(out=tp[:], in_=ps[:, :128], func=AF.Copy)
                    taps2.append(tp)
                bw2 = [load_const(f"bigw2_{t}") for t in range(3)]
                bias3 = load_const("bias3", F32)
                ps = ppool.tile([128, 512], F32, tag="ps", name="ps")
                for t in range(3):
                    nc.tensor.matmul(out=ps[:, :128], lhsT=bw2[t][:, :],
                                     rhs=taps2[t][:, :], start=(t == 0), stop=(t == 2))
                z3T = wpool.tile([128, 128], BF16, tag="z3T", name="z3T")
                nc.scalar.activation(out=z3T[:], in_=ps[:, :128], func=AF.Tanh,
                                     bias=bias3[:, 0:1])
                if debug:
                    nc.sync.dma_start(out=dbg["z3T"][:, :], in_=z3T[:])
                z3n = wpool.tile([128, 128], BF16, tag="z3n", name="z3n")
                transp(z3T[:], z3n[:])

            # ================= LEVEL 3 ====================================
            with nc.named_scope("l3"):
                s3c = load_const("s3t")
                s3sqc = load_const("s3sqt")

                def conv3(zn, zT, bwp, bias_t, func, kp):
                    taps3 = [zT]
                    for oi, opc in enumerate((s3c, s3sqc)):
                        ps = ppool.tile([128, 512], F32, tag="ps", name="ps")
                        nc.tensor.matmul(out=ps[:, :128], lhsT=zn[:], rhs=opc[:, :],
                                         start=True, stop=True)
                        tp = wpool.tile([128, 128], BF16, tag=f"{kp}t{oi}", name=f"{kp}t{oi}")
                        nc.scalar.activation(out=tp[:], in_=ps[:, :128], func=AF.Copy)
                        taps3.append(tp)
                    bw = [load_const(f"{bwp}_{t}") for t in range(3)]
                    ps = ppool.tile([128, 512], F32, tag="ps", name="ps")
                    for t in range(3):
                        nc.tensor.matmul(out=ps[:, :128], lhsT=bw[t][:, :],
                                         rhs=taps3[t][:, :], start=(t == 0), stop=(t == 2))
                    oT = wpool.tile([128, 128], BF16, tag=f"{kp}oT", name=f"{kp}oT")
                    nc.scalar.activation(out=oT[:], in_=ps[:, :128], func=func,
                                         bias=bias_t[:, 0:1])
                    on = wpool.tile([128, 128], BF16, tag=f"{kp}on", name=f"{kp}on")
                    transp(oT[:], on[:])
                    return oT, on

                bias4 = load_const("bias4", F32)
                bias5 = load_const("bias5", F32)
                z4T, z4n = conv3(z3n[:], z3T[:], "bigw3", bias4, AF.Tanh, "c4")
                z5T, _ = conv3(z4n[:], z4T[:], "bigw4", bias5, AF.Identity, "c5")

            if debug:
                nc.sync.dma_start(out=dbg["z5T"][:, :], in_=z5T[:])
            # ================= MLP input assembly =========================
            with nc.named_scope("mlp_in"):
                z5n = wpool.tile([128, 128], BF16, tag="z5n", name="z5n")
                transp(z5T[:], z5n[:])
                for b in range(4):
                    nc.sync.dma_start(
                        out=x6_loc.ap()[b:b + 1, :].rearrange("o (n h) -> n (o h)", h=32),
                        in_=z5n[:, 32 * b:32 * b + 32])
                nc.gpsimd.collective_compute(
                    "AllGather", ALU.bypass, replica_groups=RG,
                    ins=[x6_loc.ap().opt()], outs=[x6_all.ap().opt()])

            # ================= MLP ========================================
            def mlp_layer(nm, src_sb, out_sb):
                g_t = load_const("g" + nm[1], F32)
                be_t = load_const("be" + nm[1], F32)
                wts = []
                for i in range(4):
                    wt = mwpool.tile([128, 4096], BF16, tag=f"mw{i}", name=f"mw{i}")
                    nc.scalar.dma_start(out=wt[:], in_=ein[nm][:, 4096 * i:4096 * (i + 1)])
                    wts.append(wt)
                ps = ppool.tile([128, 512], F32, tag="ps", name="ps")
                for kc in range(32):
                    nc.tensor.matmul(
                        out=ps[:32, :],
                        lhsT=src_sb[:, 32 * kc:32 * kc + 32],
                        rhs=wts[kc // 8][:, 512 * (kc % 8):512 * (kc % 8 + 1)],
                        start=(kc == 0), stop=(kc == 31))
                hb = wpool.tile([32, 512], F32, tag="hb", name="hb")
                nc.scalar.activation(out=hb[:], in_=ps[:32, :], func=AF.Copy)
                for mm in range(4):
                    pst = ppool.tile([128, 512], F32, tag="ps", name="ps")
                    nc.tensor.transpose(out=pst[:128, :32],
                                        in_=hb[:, 128 * mm:128 * (mm + 1)],
                                        identity=identf32[:32, :32])
                    t = wpool.tile([128, 32], F32, tag="b_t", name="b_t")
                    nc.vector.tensor_copy(t[:], pst[:128, :32])
                    s1 = wpool.tile([128, 1], F32, tag="b_s1", name="b_s1")
                    nc.vector.tensor_reduce(out=s1[:], in_=t[:], axis=AX.X, op=ALU.add)
                    mu_ = wpool.tile([128, 1], F32, tag="b_mu", name="b_mu")
                    nc.vector.tensor_scalar_mul(mu_[:], s1[:], 1.0 / 32.0)
                    sq = wpool.tile([128, 32], F32, tag="b_sq", name="b_sq")
                    nc.vector.tensor_mul(sq[:], t[:], t[:])
                    s2_ = wpool.tile([128, 1], F32, tag="b_s2", name="b_s2")
                    nc.vector.tensor_reduce(out=s2_[:], in_=sq[:], axis=AX.X, op=ALU.add)
                    var = wpool.tile([128, 1], F32, tag="b_var", name="b_var")
                    nc.vector.scalar_tensor_tensor(out=var[:], in0=mu_[:], scalar=-1.0,
                                                   in1=mu_[:], op0=ALU.mult, op1=ALU.mult)
                    nc.vector.scalar_tensor_tensor(out=var[:], in0=s2_[:], scalar=1.0 / 32.0,
                                                   in1=var[:], op0=ALU.mult, op1=ALU.add)
                    sd = wpool.tile([128, 1], F32, tag="b_sd", name="b_sd")
                    nc.scalar.activation(out=sd[:], in_=var[:], func=AF.Sqrt,
                                         bias=eps_t[:, 0:1])
                    rs = wpool.tile([128, 1], F32, tag="b_rs", name="b_rs")
                    nc.vector.reciprocal(rs[:], sd[:])
                    a_ = wpool.tile([128, 1], F32, tag="b_a", name="b_a")
                    nc.vector.tensor_mul(a_[:], rs[:], g_t[:, mm:mm + 1])
                    sh = wpool.tile([128, 1], F32, tag="b_sh", name="b_sh")
                    nc.vector.scalar_tensor_tensor(out=sh[:], in0=mu_[:], scalar=-1.0,
                                                   in1=a_[:], op0=ALU.mult, op1=ALU.mult)
                    nc.vector.tensor_add(sh[:], sh[:], be_t[:, mm:mm + 1])
                    nc.scalar.activation(out=out_sb[:, 32 * mm:32 * mm + 32], in_=t[:],
                                         func=AF.Relu, scale=a_[:, 0:1], bias=sh[:, 0:1])

            x6T = bigpool.tile([128, 1024], BF16, tag="x6T", name="x6T")
            with nc.named_scope("mlp6"):
                xbm = wpool.tile([32, 4096], BF16, tag="xbm", name="xbm")
                nc.sync.dma_start(out=xbm[:], in_=x6_all[:, :])
                for t in range(32):
                    ps = tpool.tile([128, 128], BF16, tag="pst", name="pst")
                    nc.tensor.transpose(out=ps[:128, :32],
                                        in_=xbm[:, 128 * t:128 * (t + 1)],
                                        identity=identbf[:32, :32])
                    nc.vector.tensor_copy(x6T[:, 32 * t:32 * t + 32], ps[:128, :32])
                if debug:
                    nc.sync.dma_start(out=dbg["x6T"][:, :], in_=x6T[:])
                h6 = bigpool.tile([128, 128], BF16, tag="h6", name="h6")
                mlp_layer("w6", x6T, h6)
                if debug:
                    nc.sync.dma_start(out=dbg["h6"][:, :], in_=h6[:])
                nc.sync.dma_start(out=h_loc[6][:, :], in_=h6[:])
                nc.gpsimd.collective_compute(
                    "AllGather", ALU.bypass, replica_groups=RG,
                    ins=[h_loc[6].ap().opt()], outs=[h_all[6].ap().opt()])
            with nc.named_scope("mlp7"):
                x7T = bigpool.tile([128, 1024], BF16, tag="x7T", name="x7T")
                nc.sync.dma_start(out=x7T[:].rearrange("p (j c) -> p j c", c=128),
                                  in_=h_all[6][:, :].rearrange("(j p) c -> p j c", p=128))
                h7 = bigpool.tile([128, 128], BF16, tag="h7", name="h7")
                mlp_layer("w7", x7T, h7)
                nc.sync.dma_start(out=h_loc[7][:, :], in_=h7[:])
                nc.gpsimd.collective_compute(
                    "AllGather", ALU.bypass, replica_groups=RG,
                    ins=[h_loc[7].ap().opt()], outs=[h_all[7].ap().opt()])
            with nc.named_scope("mlp8"):
                x8T = bigpool.tile([128, 1024], BF16, tag="x8T", name="x8T")
                nc.sync.dma_start(out=x8T[:].rearrange("p (j c) -> p j c", c=128),
                                  in_=h_all[7][:, :].rearrange("(j p) c -> p j c", p=128))
                h8 = bigpool.tile([128, 128], BF16, tag="h8", name="h8")
                mlp_layer("w8", x8T, h8)

            with nc.named_scope("mlp9"):
                w9t = load_const("w9")
                ps9 = ppool.tile([128, 512], F32, tag="ps", name="ps")
                for kc in range(4):
                    nc.tensor.matmul(out=ps9[:32, :128],
                                     lhsT=h8[:, 32 * kc:32 * kc + 32],
                                     rhs=w9t[:, kc * 128:(kc + 1) * 128],
                                     start=(kc == 0), stop=(kc == 3))
                mub = wpool.tile([32, 128], F32, tag="mub", name="mub")
                nc.scalar.activation(out=mub[:], in_=ps9[:32, :128], func=AF.Copy)
                ps9t = ppool.tile([128, 512], F32, tag="ps", name="ps")
                nc.tensor.transpose(out=ps9t[:128, :32], in_=mub[:],
                                    identity=identf32[:32, :32])
                mu_sb = wpool.tile([128, 32], F32, tag="mu_sb", name="mu_sb")
                nc.scalar.activation(out=mu_sb[:], in_=ps9t[:128, :32], func=AF.Copy)
                nc.sync.dma_start(out=mu_loc[:, :], in_=mu_sb[:])
                nc.gpsimd.collective_compute(
                    "AllReduce", ALU.add, replica_groups=RG,
                    ins=[mu_loc.ap().opt()], outs=[mu_all.ap().opt()])
                tot = wpool.tile([128, 32], F32, tag="f_tot", name="f_tot")
                nc.sync.dma_start(out=tot[:], in_=mu_all[0:128, :])
                s1 = wpool.tile([128, 1], F32, tag="f_s1", name="f_s1")
                nc.vector.tensor_reduce(out=s1[:], in_=tot[:], axis=AX.X, op=ALU.add)
                mu_ = wpool.tile([128, 1], F32, tag="f_mu", name="f_mu")
                nc.vector.tensor_scalar_mul(mu_[:], s1[:], 1.0 / 32.0)
                sq = wpool.tile([128, 32], F32, tag="f_sq", name="f_sq")
                nc.vector.tensor_mul(sq[:], tot[:], tot[:])
                s2_ = wpool.tile([128, 1], F32, tag="f_s2", name="f_s2")
                nc.vector.tensor_reduce(out=s2_[:], in_=sq[:], axis=AX.X, op=ALU.add)
                var = wpool.tile([128, 1], F32, tag="f_var", name="f_var")
                nc.vector.scalar_tensor_tensor(out=var[:], in0=mu_[:], scalar=-1.0,
                                               in1=mu_[:], op0=ALU.mult, op1=ALU.mult)
                nc.vector.scalar_tensor_tensor(out=var[:], in0=s2_[:], scalar=1.0 / 32.0,
                                               in1=var[:], op0=ALU.mult, op1=ALU.add)
                sdf = wpool.tile([128, 1], F32, tag="f_sd", name="f_sd")
                nc.scalar.activation(out=sdf[:], in_=var[:], func=AF.Sqrt,
                                     bias=eps_t[:, 0:1])
                rs = wpool.tile([128, 1], F32, tag="f_rs", name="f_rs")
                nc.vector.reciprocal(rs[:], sdf[:])
                neg = wpool.tile([128, 1], F32, tag="f_neg", name="f_neg")
                nc.vector.scalar_tensor_tensor(out=neg[:], in0=mu_[:], scalar=-1.0,
                                               in1=rs[:], op0=ALU.mult, op1=ALU.mult)
                outt = wpool.tile([128, 32], F32, tag="f_out", name="f_out")
                nc.scalar.activation(out=outt[:], in_=tot[:], func=AF.Identity,
                                     scale=rs[:, 0:1], bias=neg[:, 0:1])
                nc.sync.dma_start(out=out_mu[:, :], in_=outt[:])

    nc.compile()
    return nc


# ---------------------------------------------------------------- entry point
def kernel(**inputs) -> np.ndarray:
    per_core, meta = _host_prep(inputs)
    key = (len(meta["win_of_chunk"]), tuple(meta["win_of_chunk"]),
           tuple(tuple(n) for n in meta["ncw_ops"]))
    if _CACHE.get("key") != key:
        _CACHE["prog"] = _build_nc(meta, per_core[0])
        _CACHE["key"] = key
    nc = _CACHE["prog"]
    res = bass_utils.run_bass_kernel_spmd(nc, per_core, core_ids=list(range(NCORES)))
    return np.ascontiguousarray(res.results[0]["mu"].T)
